# revision 82
# baseline (speedup 1.0000x reference)
"""Multi-head differential attention on 8 Trainium2 NeuronCores.

Sharding: data-parallel over batch (B=2) x tensor-parallel over heads
(16 heads -> 4 per core). Core c handles batch c//4 and heads
4*(c%4) .. 4*(c%4)+3. Each core computes its heads' attention output and a
partial output projection; the host sums the 4 partials per batch.

v2: bf16 operands everywhere (matmul rate is identical to f32r but DVE runs
2-4x faster and DMA halves), rope fused into the projection phase, causal
diagonal masking moved from PE matmuls to elementwise DVE multiplies, RMS
partition-reduction moved to the idle GpSimd engine, and the output
projection interleaved per 512-query block so the PE stream never drains.
"""

import math
import os
import sys

sys.path.insert(0, "/opt/trn_rl_repo")

import numpy as np

B, S, HID, NH = 2, 2048, 2048, 16
HD = HID // NH          # 128
QKD = HD // 2           # 64
NCORES = 8
GRPS = NCORES // B      # head groups per batch
HPC = NH // GRPS        # heads per core = 4
LAYER_ID = 1
LAMBDA_INIT = 0.8 - 0.6 * math.exp(-0.3 * LAYER_ID)
EPS = 1e-6

NB = S // 512           # 4 seq blocks of 512
NKC = S // 128          # 16 key chunks of 128

_PROGRAM = None         # compiled bass program, reused across calls


def _build_program():
    import concourse.bass as bass
    import concourse.tile as tile
    from concourse import bacc, bass_isa, mybir

    f32 = mybir.dt.float32
    bf16 = mybir.dt.bfloat16
    fp8 = mybir.dt.float8e4
    DR = mybir.MatmulPerfMode.DoubleRow
    Alu = mybir.AluOpType
    Act = mybir.ActivationFunctionType

    nc = bacc.Bacc(None, target_bir_lowering=False, debug=False)

    def din(name, shape, dt=bf16):
        return nc.dram_tensor(name, shape, dt, kind="ExternalInput").ap()

    io = {
        "xq_t": din("xq_t", [HID, S]),
        "xk_t": din("xk_t", [HID, S]),
        "xv_t": din("xv_t", [HID, S]),
        "wq_t": din("wq_t", [HID, 512]),
        "wk_t": din("wk_t", [HID, 512]),
        "wv_t": din("wv_t", [HID, 512]),
        "wo_t": din("wo_t", [512, HID]),
        "crep": din("crep", [128, S]),
        "srep": din("srep", [128, S]),
        "pmat": din("pmat", [128, 128]),
        "ones_a": din("ones_a", [128, 128]),
        "trimask": din("trimask", [128, 128]),
        "neglam": din("neglam", [128, 1], f32),
    }
    y_t = nc.dram_tensor("y_t", [HID, S], f32, kind="ExternalOutput").ap()

    from contextlib import ExitStack

    with tile.TileContext(nc) as tc, ExitStack() as ctx:
        persist = ctx.enter_context(tc.tile_pool(name="persist", bufs=1))
        constp = ctx.enter_context(tc.tile_pool(name="constp", bufs=1))

        # constants
        crep = constp.tile([128, S], bf16, name="crep_sb", tag="crep")
        srep = constp.tile([128, S], bf16, name="srep_sb", tag="srep")
        pmat = constp.tile([128, 128], bf16, name="pmat_sb", tag="pmat")
        ones_a = constp.tile([128, 128], bf16, name="ones_a_sb", tag="ones_a")
        trimask = constp.tile([128, 128], bf16, name="trimask_sb", tag="trimask")
        neglam = constp.tile([128, 1], f32, name="neglam_sb", tag="neglam")
        epsb = constp.tile([128, 1], f32, name="epsb", tag="epsb")
        fones = constp.tile([128, 1024], f32, name="fones", tag="fones")
        nc.vector.memset(epsb[:], EPS)
        nc.vector.memset(fones[:], 1.0)

        # persistent tensors: Q^T/K^T per (map g, head-pair hp): [128, S]
        #   tile t = 2*g + hp; partitions [64*a, 64*a+64) hold head 2*hp+a.
        QT = [persist.tile([128, S], bf16, name=f"qt{t}", tag=f"qt{t}")
              for t in range(4)]
        KT = [persist.tile([128, S], bf16, name=f"kt{t}", tag=f"kt{t}")
              for t in range(4)]
        # V natural layout per 128-seq chunk: [128 seq, 4 heads * 128 feat]
        VH = [persist.tile([128, 512], bf16, name=f"vh{s}", tag=f"vh{s}")
              for s in range(NKC)]
        # combined attention output (post RMS): [feat, seq] per head
        U = [persist.tile([128, S], bf16, name=f"u{h}", tag=f"u{h}")
             for h in range(HPC)]
        wo = [persist.tile([128, S], bf16, name=f"wo{h}", tag=f"wo{h}")
              for h in range(HPC)]

        # ---------------- phase P: q/k/v projections + fused rope ----------
        with tc.tile_pool(name="wp", bufs=1) as wp, \
             tc.tile_pool(name="xp", bufs=3) as xp, \
             tc.tile_pool(name="rsb", bufs=2) as rsb, \
             tc.tile_pool(name="pp", bufs=1, space="PSUM") as pp, \
             tc.tile_pool(name="pxp", bufs=2, space="PSUM") as pxp:
            wt = {}
            wnames = ("wq_t", "wk_t", "wv_t")

            def load_weights(mi):
                # distinct tags per mode: a later mode's weight DMAs must not
                # sit WAR-blocked in front of that mode's x transfers in the
                # in-order DMA queues
                for kc in range(NKC):
                    w_ = wp.tile([128, 512], bf16, name=f"w{mi}_{kc}",
                                 tag=f"w{mi}_{kc}")
                    nc.sync.dma_start(
                        out=w_[:], in_=io[wnames[mi]][kc * 128:(kc + 1) * 128, :])
                    wt[(mi, kc)] = w_

            # critical-path DMAs first: wq chunks + the first x block feed
            # the very first matmuls; big constants (crep/srep/wo) are not
            # needed until rope (~20us) / attention (~200us) and would
            # otherwise delay the first MULT by ~18us in queue order
            load_weights(0)
            xpre = {}
            for n in (0, 1, 2):
                for kc in range(NKC):
                    xck = xp.tile([128, 512], bf16, name=f"x_0_{n}_{kc}",
                                  tag=f"x{kc}")
                    nc.sync.dma_start(
                        out=xck[:],
                        in_=io["xq_t"][kc * 128:(kc + 1) * 128,
                                       n * 512:(n + 1) * 512])
                    xpre.setdefault(n, []).append(xck)
            for t, key in ((pmat, "pmat"), (trimask, "trimask"),
                           (neglam, "neglam"), (ones_a, "ones_a"),
                           (crep, "crep"), (srep, "srep")):
                nc.sync.dma_start(out=t[:], in_=io[key][:])
            for h in range(HPC):
                nc.sync.dma_start(out=wo[h][:],
                                  in_=io["wo_t"][h * 128:(h + 1) * 128, :])

            pending = []

            def flush_rope():
                T, t, n, raw = pending.pop()
                px = pxp.tile([128, 512], f32, name=f"px_{T[t].name}_{n}",
                              tag="px")
                nc.tensor.matmul(px[:], pmat[:], raw[:], start=True, stop=True)
                pxb = rsb.tile([128, 512], bf16, name=f"pxb_{T[t].name}_{n}",
                               tag="pxb")
                nc.scalar.copy(pxb[:], px[:])
                cs = slice(n * 512, (n + 1) * 512)
                tmp = rsb.tile([128, 512], bf16, name=f"tmp_{T[t].name}_{n}",
                               tag="tmp")
                nc.vector.tensor_mul(tmp[:], pxb[:], srep[:, cs])
                aa = rsb.tile([128, 512], bf16, name=f"aa_{T[t].name}_{n}",
                              tag="aa")
                nc.vector.tensor_mul(aa[:], raw[:], crep[:, cs])
                nc.vector.tensor_add(T[t][:, cs], aa[:], tmp[:])

            for mi in range(3):
                xin = io[("xq_t", "xk_t", "xv_t")[mi]]
                for n in range(NB):
                    if mi == 0 and n in xpre:
                        xt = xpre[n]
                    else:
                        xt = []
                        for kc in range(NKC):
                            xck = xp.tile([128, 512], bf16,
                                          name=f"x_{mi}_{n}_{kc}",
                                          tag=f"x{kc}")
                            nc.sync.dma_start(
                                out=xck[:],
                                in_=xin[kc * 128:(kc + 1) * 128,
                                        n * 512:(n + 1) * 512])
                            xt.append(xck)
                    if n == 0 and mi < 2:
                        load_weights(mi + 1)   # prefetch next mode's weights
                    for t in range(4):
                        ps = pp.tile([128, 512], f32, name=f"pp{t}_{mi}_{n}",
                                     tag=f"pp{t}")
                        for kc in range(NKC):
                            nc.tensor.matmul(ps[:],
                                             wt[(mi, kc)][:, t * 128:(t + 1) * 128]
                                             if mi < 2 else
                                             xt[kc][:, t * 128:(t + 1) * 128],
                                             xt[kc][:]
                                             if mi < 2 else wt[(mi, kc)][:],
                                             start=(kc == 0), stop=(kc == 15))
                            if kc == 2 and pending:
                                flush_rope()
                        if mi == 2:
                            nc.scalar.copy(VH[n * 4 + t][:], ps[:])
                        else:
                            raw = rsb.tile([128, 512], bf16,
                                           name=f"raw_{mi}_{n}_{t}", tag="raw")
                            nc.vector.tensor_copy(raw[:], ps[:])
                            pending.append((QT if mi == 0 else KT, t, n, raw))
            while pending:
                flush_rope()

        # ---------------- phase A: attention + rms + output proj ----------
        with tc.tile_pool(name="sp", bufs=2, space="PSUM") as sp, \
             tc.tile_pool(name="pvp", bufs=1, space="PSUM") as pvp, \
             tc.tile_pool(name="smp", bufs=1, space="PSUM") as smp, \
             tc.tile_pool(name="ep", bufs=5) as ep, \
             tc.tile_pool(name="cb", bufs=2) as cb, \
             tc.tile_pool(name="ys", bufs=2) as ys:
            pending_rms = []
            i32 = mybir.dt.int32

            def flush_rms():
                # rms tail for a completed (h, qb): partition-sum of U^2 on
                # the PE, then rstd = rsqrt(mean+eps) via the exponent
                # bit-trick + one Newton step on the vector engine. Using Ln
                # on the scalar engine would force an activation-table reload
                # (the chooser picks the exp-less natural_log set) costing
                # 2x1.28us per head; Copy is resident in every table.
                h_, qb_, sq_, dst_ = pending_rms.pop()
                ssq_t = sp.tile([128, 1024], f32, name=f"ssq_{h_}_{qb_}",
                                tag="s")
                nc.tensor.matmul(ssq_t[:, 0:512], ones_a[:], sq_[:],
                                 start=True, stop=True)
                m_ = cb.tile([128, 512], f32, name=f"m_{h_}_{qb_}", tag="m")
                nc.scalar.activation(m_[:], ssq_t[:, 0:512], Act.Copy,
                                     bias=EPS, scale=1.0 / HD)
                y0 = cb.tile([128, 512], f32, name=f"y0_{h_}_{qb_}", tag="y0")
                nc.vector.tensor_scalar(
                    y0[:].bitcast(i32), m_[:].bitcast(i32), 1, -1,
                    op0=Alu.logical_shift_right, op1=Alu.bitwise_xor)
                nc.vector.tensor_scalar(
                    y0[:].bitcast(i32), y0[:].bitcast(i32), 0x5f3759e0, None,
                    op0=Alu.add)
                uu = cb.tile([128, 512], f32, name=f"uu_{h_}_{qb_}", tag="uu")
                nc.vector.tensor_mul(uu[:], y0[:], y0[:])
                nc.vector.scalar_tensor_tensor(
                    uu[:], m_[:], -0.5, uu[:], op0=Alu.mult, op1=Alu.mult)
                nc.vector.tensor_scalar(uu[:], uu[:], 1.5, None, op0=Alu.add)
                rstdb = cb.tile([128, 512], bf16, name=f"rstdb_{h_}_{qb_}",
                                tag="rstdb")
                nc.vector.tensor_mul(rstdb[:], y0[:], uu[:])
                nc.vector.tensor_mul(dst_, dst_, rstdb[:])

            pending_y = None

            def emit_y(qb, final=False):
                # output projection for query block qb; emitted one head into
                # block qb+1 so the last head's rms tail is long finished.
                # For the final block nothing else hides that tail, so emit
                # h0-h2 contributions of two chains first, h3 ones after.
                if final:
                    for grp in range(4):
                        tiles = []
                        for pr in (2 * grp, 2 * grp + 1):
                            psy = sp.tile([128, 1024], f32,
                                          name=f"py_{qb}_{pr}", tag="s")
                            for i in (0, 1):
                                oc = 2 * pr + i
                                for h2 in range(HPC - 1):
                                    nc.tensor.matmul(
                                        psy[:, i * 512:(i + 1) * 512],
                                        wo[h2][:, oc * 128:(oc + 1) * 128],
                                        U[h2][:, qb * 512:(qb + 1) * 512],
                                        start=(h2 == 0), stop=False)
                            tiles.append((pr, psy))
                        for pr, psy in tiles:
                            for i in (0, 1):
                                oc = 2 * pr + i
                                nc.tensor.matmul(
                                    psy[:, i * 512:(i + 1) * 512],
                                    wo[HPC - 1][:, oc * 128:(oc + 1) * 128],
                                    U[HPC - 1][:, qb * 512:(qb + 1) * 512],
                                    start=False, stop=True)
                            yst = ys.tile([128, 1024], f32,
                                          name=f"yst_{qb}_{pr}", tag="yst")
                            nc.scalar.copy(yst[:], psy[:])
                            for i in (0, 1):
                                oc = 2 * pr + i
                                nc.sync.dma_start(
                                    out=y_t[oc * 128:(oc + 1) * 128,
                                            qb * 512:(qb + 1) * 512],
                                    in_=yst[:, i * 512:(i + 1) * 512])
                    return
                for pr in range(8):
                    psy = sp.tile([128, 1024], f32, name=f"py_{qb}_{pr}",
                                  tag="s")
                    for i in (0, 1):
                        oc = 2 * pr + i
                        for h2 in range(HPC):
                            nc.tensor.matmul(
                                psy[:, i * 512:(i + 1) * 512],
                                wo[h2][:, oc * 128:(oc + 1) * 128],
                                U[h2][:, qb * 512:(qb + 1) * 512],
                                start=(h2 == 0), stop=(h2 == HPC - 1))
                    yst = ys.tile([128, 1024], f32, name=f"yst_{qb}_{pr}",
                                  tag="yst")
                    nc.scalar.copy(yst[:], psy[:])
                    for i in (0, 1):
                        oc = 2 * pr + i
                        nc.sync.dma_start(
                            out=y_t[oc * 128:(oc + 1) * 128,
                                    qb * 512:(qb + 1) * 512],
                            in_=yst[:, i * 512:(i + 1) * 512])

            for qb in range(NB):
                for h in range(HPC):
                    hp, a = h // 2, h % 2
                    poff = 64 * a
                    pv = pvp.tile([128, 1024], f32, name=f"pv_{h}_{qb}",
                                  tag="pv")
                    sm = smp.tile([128, 1024], f32, name=f"sm_{h}_{qb}",
                                  tag="sm")
                    nkc = 4 * qb + 4
                    pvq = []   # (E, qoff, first, last, kc) pending PV/SM

                    def emit_pv_sm(item):
                        E_, qoff_, first_, last_, kc_ = item
                        for g in (0, 1):
                            sl = slice(g * 512 + qoff_, (g + 1) * 512)
                            nc.tensor.matmul(
                                pv[:, sl],
                                VH[kc_][:, h * 128:(h + 1) * 128],
                                E_[:, sl], start=first_, stop=last_)
                            nc.tensor.matmul(
                                sm[:, sl], ones_a[:], E_[:, sl],
                                start=first_, stop=last_)

                    for kc in range(nkc):
                        j = kc - 4 * qb
                        qoff = max(j, 0) * 128
                        ps = sp.tile([128, 1024], f32, name=f"s_{h}_{qb}_{kc}",
                                     tag="s")
                        for g in (0, 1):
                            tq = 2 * g + hp
                            nc.tensor.matmul(
                                ps[:, g * 512 + qoff:(g + 1) * 512],
                                KT[tq][poff:poff + 64,
                                       kc * 128:(kc + 1) * 128],
                                QT[tq][poff:poff + 64,
                                       qb * 512 + qoff:(qb + 1) * 512],
                                start=True, stop=True)
                        E = ep.tile([128, 1024], bf16, name=f"e_{h}_{qb}_{kc}",
                                    tag="e")
                        if qoff == 0:
                            nc.scalar.activation(E[:], ps[:], Act.Exp,
                                                 scale=0.125)
                        else:
                            for g in (0, 1):
                                nc.scalar.activation(
                                    E[:, g * 512 + qoff:(g + 1) * 512],
                                    ps[:, g * 512 + qoff:(g + 1) * 512],
                                    Act.Exp, scale=0.125)
                        if j >= 0:
                            for g in (0, 1):
                                sl = E[:, g * 512 + qoff:g * 512 + qoff + 128]
                                # causal mask on the idle GpSimd engine (all
                                # SBUF operands): keeps the mask->PV latency
                                # off the vector queue, where the rsqrt chain
                                # flushed at kc==3 would delay qb=1's diagonal
                                # chunks
                                nc.gpsimd.tensor_mul(sl, sl, trimask[:])
                        if kc == 3 and pending_rms:
                            flush_rms()
                        pvq.append((E, qoff, kc == 0, kc == nkc - 1, kc))
                        if len(pvq) > 3:
                            emit_pv_sm(pvq.pop(0))
                    while pvq:
                        emit_pv_sm(pvq.pop(0))
                    # combine: U = pv1/sm1 - lam * pv2/sm2, then RMS norm
                    rb = cb.tile([128, 1024], f32, name=f"rb_{h}_{qb}",
                                 tag="rb")
                    nc.vector.reciprocal_approx_fast(rb[:], sm[:])
                    tt = cb.tile([128, 1024], f32, name=f"tt_{h}_{qb}",
                                 tag="tt")
                    nc.vector.tensor_mul(tt[:], pv[:], rb[:])
                    dst = U[h][:, qb * 512:(qb + 1) * 512]
                    nc.vector.scalar_tensor_tensor(
                        dst, tt[:, 512:1024], neglam[:], tt[:, 0:512],
                        op0=Alu.mult, op1=Alu.add)
                    sq = cb.tile([128, 512], bf16, name=f"sq_{h}_{qb}",
                                 tag="sq")
                    nc.vector.tensor_mul(sq[:], dst, dst)
                    pending_rms.append((h, qb, sq, dst))
                    if h == 0 and pending_y is not None:
                        emit_y(pending_y)
                        pending_y = None
                pending_y = qb
            while pending_rms:
                flush_rms()
            emit_y(pending_y, final=True)

    nc.compile()
    return nc


def _host_prep(q, k, v, Wq, Wk, Wv, Wo, lambda_q1, lambda_k1, lambda_q2,
               lambda_k2, gnorm_w, cos_emb, sin_emb):
    import ml_dtypes

    f32 = np.float32
    bf16 = ml_dtypes.bfloat16
    q = np.asarray(q, f32); k = np.asarray(k, f32); v = np.asarray(v, f32)
    Wq = np.asarray(Wq, f32); Wk = np.asarray(Wk, f32)
    Wv = np.asarray(Wv, f32); Wo = np.asarray(Wo, f32)
    gnorm_w = np.asarray(gnorm_w, f32)
    cos_emb = np.asarray(cos_emb, f32); sin_emb = np.asarray(sin_emb, f32)

    lam1 = np.exp(np.sum(np.asarray(lambda_q1, f32) * np.asarray(lambda_k1, f32),
                         dtype=f32))
    lam2 = np.exp(np.sum(np.asarray(lambda_q2, f32) * np.asarray(lambda_k2, f32),
                         dtype=f32))
    lam = np.float32(lam1 - lam2 + LAMBDA_INIT)

    # per-batch transposed activations (bf16)
    xt = {}
    for b in range(B):
        xt[("q", b)] = np.ascontiguousarray(q[b].T).astype(bf16)
        xt[("k", b)] = np.ascontiguousarray(k[b].T).astype(bf16)
        xt[("v", b)] = np.ascontiguousarray(v[b].T).astype(bf16)

    # shared constant tensors
    base_c = cos_emb[:S, :QKD]          # [S, 64]
    base_s = sin_emb[:S, :QKD]
    crep = np.ascontiguousarray(np.tile(base_c.T, (2, 1))).astype(bf16)
    srep = np.ascontiguousarray(np.tile(base_s.T, (2, 1))).astype(bf16)
    pmat = np.zeros((128, 128), f32)
    for blk in range(2):
        o = blk * 64
        for i in range(QKD // 2):
            pmat[o + 2 * i, o + 2 * i + 1] = 1.0     # lhsT[2i, 2i+1]
            pmat[o + 2 * i + 1, o + 2 * i] = -1.0    # lhsT[2i+1, 2i]
    pmat = pmat.astype(bf16)
    ones_a = np.ones((128, 128), f32).astype(bf16)
    # trimask[p, n] = 1 if key-in-chunk p <= query-in-block n (valid)
    trimask = np.triu(np.ones((128, 128), f32), 0).astype(bf16)
    neglam = np.full((128, 1), -lam, f32)

    per_core = []
    for c in range(NCORES):
        b, grp = c // GRPS, c % GRPS
        heads = [HPC * grp + j for j in range(HPC)]
        # wq/wk columns: tile t = 2*g + hp; within tile: head 2*hp+a at
        # cols [64*a, 64*a+64), original feature order (interleaved pairs)
        cols = []
        for t in range(4):
            g, hp = t // 2, t % 2
            for a2 in range(2):
                hg = heads[2 * hp + a2]
                cols.extend(hg * HD + g * QKD + d for d in range(QKD))
        cols = np.asarray(cols)
        vrows = np.asarray([h * HD + d for h in heads for d in range(HD)])
        wq_t = np.ascontiguousarray(Wq[cols, :].T).astype(bf16)
        wk_t = np.ascontiguousarray(Wk[cols, :].T).astype(bf16)
        wv_t = np.ascontiguousarray(Wv[vrows, :].T).astype(bf16)
        gtile = np.tile(gnorm_w, HPC)                       # [512]
        wo_t = np.ascontiguousarray(
            ((1.0 - LAMBDA_INIT) * Wo[:, vrows] * gtile[None, :]).T).astype(bf16)
        per_core.append({
            "xq_t": xt[("q", b)], "xk_t": xt[("k", b)], "xv_t": xt[("v", b)],
            "wq_t": wq_t, "wk_t": wk_t, "wv_t": wv_t, "wo_t": wo_t,
            "crep": crep, "srep": srep, "pmat": pmat,
            "ones_a": ones_a, "trimask": trimask, "neglam": neglam,
        })
    return per_core


def _install_ntff_hook():
    """antenv.axon_hooks is absent in this image; synthesize it so
    run_bass_kernel_spmd(trace=True) can capture NTFF profiles."""
    import sys as _sys
    import types

    if "antenv.axon_hooks" in _sys.modules:
        return
    import antenv
    mod = types.ModuleType("antenv.axon_hooks")
    state = {"hook": None}
    mod.set_axon_ntff_profile_hook = lambda h: state.__setitem__("hook", h)
    mod.get_axon_ntff_profile_hook = lambda: state["hook"]
    _sys.modules["antenv.axon_hooks"] = mod
    antenv.axon_hooks = mod
    try:
        from trn_agent_boot.trn_boot import _ntff_profile_via_ctypes
        state["hook"] = _ntff_profile_via_ctypes("/opt/axon/libaxon_pjrt.so")
    except Exception as e:  # degrade: trace skipped, run still works
        print("ntff hook install failed:", e)


def kernel(q, k, v, Wq, Wk, Wv, Wo, lambda_q1, lambda_k1, lambda_q2,
           lambda_k2, gnorm_w, cos_emb, sin_emb, mask, _trace=False):
    if _trace:
        _install_ntff_hook()
    global _PROGRAM
    if _PROGRAM is None:
        _PROGRAM = _build_program()
    nc = _PROGRAM

    in_maps = _host_prep(q, k, v, Wq, Wk, Wv, Wo, lambda_q1, lambda_k1,
                         lambda_q2, lambda_k2, gnorm_w, cos_emb, sin_emb)

    from concourse.bass_utils import run_bass_kernel_spmd
    res = run_bass_kernel_spmd(nc, in_maps, core_ids=list(range(NCORES)),
                               trace=_trace)
    kernel.last_result = res

    y = np.zeros((B, S, HID), np.float32)
    for c in range(NCORES):
        y[c // GRPS] += res.results[c]["y_t"].T
    return y


# revision 84
# speedup vs baseline: 1.0030x; 1.0030x over previous
"""Multi-head differential attention on 8 Trainium2 NeuronCores.

Sharding: data-parallel over batch (B=2) x tensor-parallel over heads
(16 heads -> 4 per core). Core c handles batch c//4 and heads
4*(c%4) .. 4*(c%4)+3. Each core computes its heads' attention output and a
partial output projection; the host sums the 4 partials per batch.

v2: bf16 operands everywhere (matmul rate is identical to f32r but DVE runs
2-4x faster and DMA halves), rope fused into the projection phase, causal
diagonal masking moved from PE matmuls to elementwise DVE multiplies, RMS
partition-reduction moved to the idle GpSimd engine, and the output
projection interleaved per 512-query block so the PE stream never drains.
"""

import math
import os
import sys

sys.path.insert(0, "/opt/trn_rl_repo")

import numpy as np

B, S, HID, NH = 2, 2048, 2048, 16
HD = HID // NH          # 128
QKD = HD // 2           # 64
NCORES = 8
GRPS = NCORES // B      # head groups per batch
HPC = NH // GRPS        # heads per core = 4
LAYER_ID = 1
LAMBDA_INIT = 0.8 - 0.6 * math.exp(-0.3 * LAYER_ID)
EPS = 1e-6

NB = S // 512           # 4 seq blocks of 512
NKC = S // 128          # 16 key chunks of 128

_PROGRAM = None         # compiled bass program, reused across calls


def _build_program():
    import concourse.bass as bass
    import concourse.tile as tile
    from concourse import bacc, bass_isa, mybir

    f32 = mybir.dt.float32
    bf16 = mybir.dt.bfloat16
    fp8 = mybir.dt.float8e4
    DR = mybir.MatmulPerfMode.DoubleRow
    Alu = mybir.AluOpType
    Act = mybir.ActivationFunctionType

    nc = bacc.Bacc(None, target_bir_lowering=False, debug=False)

    def din(name, shape, dt=bf16):
        return nc.dram_tensor(name, shape, dt, kind="ExternalInput").ap()

    io = {
        "xq_t": din("xq_t", [HID, S]),
        "xk_t": din("xk_t", [HID, S]),
        "xv_t": din("xv_t", [HID, S]),
        "wq_t": din("wq_t", [HID, 512]),
        "wk_t": din("wk_t", [HID, 512]),
        "wv_t": din("wv_t", [HID, 512]),
        "wo_t": din("wo_t", [512, HID]),
        "crep": din("crep", [128, S]),
        "srep": din("srep", [128, S]),
        "pmat": din("pmat", [128, 128]),
        "ones_a": din("ones_a", [128, 128]),
        "trimask": din("trimask", [128, 128]),
        "neglam": din("neglam", [128, 1], f32),
    }
    y_t = nc.dram_tensor("y_t", [HID, S], f32, kind="ExternalOutput").ap()

    from contextlib import ExitStack

    with tile.TileContext(nc) as tc, ExitStack() as ctx:
        persist = ctx.enter_context(tc.tile_pool(name="persist", bufs=1))
        constp = ctx.enter_context(tc.tile_pool(name="constp", bufs=1))

        # constants
        crep = constp.tile([128, S], bf16, name="crep_sb", tag="crep")
        srep = constp.tile([128, S], bf16, name="srep_sb", tag="srep")
        pmat = constp.tile([128, 128], bf16, name="pmat_sb", tag="pmat")
        ones_a = constp.tile([128, 128], bf16, name="ones_a_sb", tag="ones_a")
        trimask = constp.tile([128, 128], bf16, name="trimask_sb", tag="trimask")
        neglam = constp.tile([128, 1], f32, name="neglam_sb", tag="neglam")
        epsb = constp.tile([128, 1], f32, name="epsb", tag="epsb")
        fones = constp.tile([128, 1024], f32, name="fones", tag="fones")
        nc.vector.memset(epsb[:], EPS)
        nc.vector.memset(fones[:], 1.0)

        # persistent tensors: Q^T/K^T per (map g, head-pair hp): [128, S]
        #   tile t = 2*g + hp; partitions [64*a, 64*a+64) hold head 2*hp+a.
        QT = [persist.tile([128, S], bf16, name=f"qt{t}", tag=f"qt{t}")
              for t in range(4)]
        KT = [persist.tile([128, S], bf16, name=f"kt{t}", tag=f"kt{t}")
              for t in range(4)]
        # V natural layout per 128-seq chunk: [128 seq, 4 heads * 128 feat]
        VH = [persist.tile([128, 512], bf16, name=f"vh{s}", tag=f"vh{s}")
              for s in range(NKC)]
        # combined attention output (post RMS): [feat, seq] per head
        U = [persist.tile([128, S], bf16, name=f"u{h}", tag=f"u{h}")
             for h in range(HPC)]
        wo = [persist.tile([128, S], bf16, name=f"wo{h}", tag=f"wo{h}")
              for h in range(HPC)]

        # ---------------- phase P: q/k/v projections + fused rope ----------
        with tc.tile_pool(name="wp", bufs=1) as wp, \
             tc.tile_pool(name="xp", bufs=2) as xp, \
             tc.tile_pool(name="rsb", bufs=2) as rsb, \
             tc.tile_pool(name="pp", bufs=1, space="PSUM") as pp, \
             tc.tile_pool(name="pxp", bufs=2, space="PSUM") as pxp:
            wt = {}
            wnames = ("wq_t", "wk_t", "wv_t")

            def load_weights(mi):
                # distinct tags per mode: a later mode's weight DMAs must not
                # sit WAR-blocked in front of that mode's x transfers in the
                # in-order DMA queues
                for kc in range(NKC):
                    w_ = wp.tile([128, 512], bf16, name=f"w{mi}_{kc}",
                                 tag=f"w{mi}_{kc}")
                    nc.sync.dma_start(
                        out=w_[:], in_=io[wnames[mi]][kc * 128:(kc + 1) * 128, :])
                    wt[(mi, kc)] = w_

            # critical-path DMAs first: wq chunks + the first x block feed
            # the very first matmuls; big constants (crep/srep/wo) are not
            # needed until rope (~20us) / attention (~200us) and would
            # otherwise delay the first MULT by ~18us in queue order
            load_weights(0)
            xpre = {}
            for n in (0, 1):
                for kc in range(NKC):
                    xck = xp.tile([128, 512], bf16, name=f"x_0_{n}_{kc}",
                                  tag=f"x{kc}")
                    nc.sync.dma_start(
                        out=xck[:],
                        in_=io["xq_t"][kc * 128:(kc + 1) * 128,
                                       n * 512:(n + 1) * 512])
                    xpre.setdefault(n, []).append(xck)
            for t, key in ((pmat, "pmat"), (trimask, "trimask"),
                           (neglam, "neglam"), (ones_a, "ones_a"),
                           (crep, "crep"), (srep, "srep")):
                nc.sync.dma_start(out=t[:], in_=io[key][:])
            for h in range(HPC):
                nc.sync.dma_start(out=wo[h][:],
                                  in_=io["wo_t"][h * 128:(h + 1) * 128, :])

            pending = []

            def flush_rope():
                T, t, n, raw = pending.pop()
                px = pxp.tile([128, 512], f32, name=f"px_{T[t].name}_{n}",
                              tag="px")
                nc.tensor.matmul(px[:], pmat[:], raw[:], start=True, stop=True)
                pxb = rsb.tile([128, 512], bf16, name=f"pxb_{T[t].name}_{n}",
                               tag="pxb")
                nc.scalar.copy(pxb[:], px[:])
                cs = slice(n * 512, (n + 1) * 512)
                tmp = rsb.tile([128, 512], bf16, name=f"tmp_{T[t].name}_{n}",
                               tag="tmp")
                nc.vector.tensor_mul(tmp[:], pxb[:], srep[:, cs])
                aa = rsb.tile([128, 512], bf16, name=f"aa_{T[t].name}_{n}",
                              tag="aa")
                nc.vector.tensor_mul(aa[:], raw[:], crep[:, cs])
                nc.vector.tensor_add(T[t][:, cs], aa[:], tmp[:])

            for mi in range(3):
                xin = io[("xq_t", "xk_t", "xv_t")[mi]]
                for n in range(NB):
                    if mi == 0 and n in xpre:
                        xt = xpre[n]
                    else:
                        xt = []
                        for kc in range(NKC):
                            xck = xp.tile([128, 512], bf16,
                                          name=f"x_{mi}_{n}_{kc}",
                                          tag=f"x{kc}")
                            nc.sync.dma_start(
                                out=xck[:],
                                in_=xin[kc * 128:(kc + 1) * 128,
                                        n * 512:(n + 1) * 512])
                            xt.append(xck)
                    if n == 0 and mi < 2:
                        load_weights(mi + 1)   # prefetch next mode's weights
                    for t in range(4):
                        ps = pp.tile([128, 512], f32, name=f"pp{t}_{mi}_{n}",
                                     tag=f"pp{t}")
                        for kc in range(NKC):
                            nc.tensor.matmul(ps[:],
                                             wt[(mi, kc)][:, t * 128:(t + 1) * 128]
                                             if mi < 2 else
                                             xt[kc][:, t * 128:(t + 1) * 128],
                                             xt[kc][:]
                                             if mi < 2 else wt[(mi, kc)][:],
                                             start=(kc == 0), stop=(kc == 15))
                            if kc == 2 and pending:
                                flush_rope()
                        if mi == 2:
                            nc.scalar.copy(VH[n * 4 + t][:], ps[:])
                        else:
                            raw = rsb.tile([128, 512], bf16,
                                           name=f"raw_{mi}_{n}_{t}", tag="raw")
                            # on scalar: in the vector queue this cast sits
                            # behind ~5us of rope multiplies, and the next
                            # block's psum-tag WAR stalls the PE on it
                            nc.scalar.copy(raw[:], ps[:])
                            pending.append((QT if mi == 0 else KT, t, n, raw))
            while pending:
                flush_rope()

        # ---------------- phase A: attention + rms + output proj ----------
        with tc.tile_pool(name="sp", bufs=2, space="PSUM") as sp, \
             tc.tile_pool(name="pvp", bufs=1, space="PSUM") as pvp, \
             tc.tile_pool(name="smp", bufs=1, space="PSUM") as smp, \
             tc.tile_pool(name="ep", bufs=5) as ep, \
             tc.tile_pool(name="cb", bufs=2) as cb, \
             tc.tile_pool(name="ys", bufs=2) as ys:
            pending_rms = []
            i32 = mybir.dt.int32

            def flush_rms():
                # rms tail for a completed (h, qb): partition-sum of U^2 on
                # the PE, then rstd = rsqrt(mean+eps) via the exponent
                # bit-trick + one Newton step on the vector engine. Using Ln
                # on the scalar engine would force an activation-table reload
                # (the chooser picks the exp-less natural_log set) costing
                # 2x1.28us per head; Copy is resident in every table.
                h_, qb_, sq_, dst_ = pending_rms.pop()
                ssq_t = sp.tile([128, 1024], f32, name=f"ssq_{h_}_{qb_}",
                                tag="s")
                nc.tensor.matmul(ssq_t[:, 0:512], ones_a[:], sq_[:],
                                 start=True, stop=True)
                m_ = cb.tile([128, 512], f32, name=f"m_{h_}_{qb_}", tag="m")
                nc.scalar.activation(m_[:], ssq_t[:, 0:512], Act.Copy,
                                     bias=EPS, scale=1.0 / HD)
                y0 = cb.tile([128, 512], f32, name=f"y0_{h_}_{qb_}", tag="y0")
                nc.vector.tensor_scalar(
                    y0[:].bitcast(i32), m_[:].bitcast(i32), 1, -1,
                    op0=Alu.logical_shift_right, op1=Alu.bitwise_xor)
                nc.vector.tensor_scalar(
                    y0[:].bitcast(i32), y0[:].bitcast(i32), 0x5f3759e0, None,
                    op0=Alu.add)
                uu = cb.tile([128, 512], f32, name=f"uu_{h_}_{qb_}", tag="uu")
                nc.vector.tensor_mul(uu[:], y0[:], y0[:])
                nc.vector.scalar_tensor_tensor(
                    uu[:], m_[:], -0.5, uu[:], op0=Alu.mult, op1=Alu.mult)
                nc.vector.tensor_scalar(uu[:], uu[:], 1.5, None, op0=Alu.add)
                rstdb = cb.tile([128, 512], bf16, name=f"rstdb_{h_}_{qb_}",
                                tag="rstdb")
                nc.vector.tensor_mul(rstdb[:], y0[:], uu[:])
                nc.vector.tensor_mul(dst_, dst_, rstdb[:])

            pending_y = None

            def emit_y(qb, final=False):
                # output projection for query block qb; emitted one head into
                # block qb+1 so the last head's rms tail is long finished.
                # For the final block nothing else hides that tail, so emit
                # h0-h2 contributions of two chains first, h3 ones after.
                if final:
                    for grp in range(4):
                        tiles = []
                        for pr in (2 * grp, 2 * grp + 1):
                            psy = sp.tile([128, 1024], f32,
                                          name=f"py_{qb}_{pr}", tag="s")
                            for i in (0, 1):
                                oc = 2 * pr + i
                                for h2 in range(HPC - 1):
                                    nc.tensor.matmul(
                                        psy[:, i * 512:(i + 1) * 512],
                                        wo[h2][:, oc * 128:(oc + 1) * 128],
                                        U[h2][:, qb * 512:(qb + 1) * 512],
                                        start=(h2 == 0), stop=False)
                            tiles.append((pr, psy))
                        for pr, psy in tiles:
                            for i in (0, 1):
                                oc = 2 * pr + i
                                nc.tensor.matmul(
                                    psy[:, i * 512:(i + 1) * 512],
                                    wo[HPC - 1][:, oc * 128:(oc + 1) * 128],
                                    U[HPC - 1][:, qb * 512:(qb + 1) * 512],
                                    start=False, stop=True)
                            yst = ys.tile([128, 1024], f32,
                                          name=f"yst_{qb}_{pr}", tag="yst")
                            nc.scalar.copy(yst[:], psy[:])
                            for i in (0, 1):
                                oc = 2 * pr + i
                                nc.sync.dma_start(
                                    out=y_t[oc * 128:(oc + 1) * 128,
                                            qb * 512:(qb + 1) * 512],
                                    in_=yst[:, i * 512:(i + 1) * 512])
                    return
                for pr in range(8):
                    psy = sp.tile([128, 1024], f32, name=f"py_{qb}_{pr}",
                                  tag="s")
                    for i in (0, 1):
                        oc = 2 * pr + i
                        for h2 in range(HPC):
                            nc.tensor.matmul(
                                psy[:, i * 512:(i + 1) * 512],
                                wo[h2][:, oc * 128:(oc + 1) * 128],
                                U[h2][:, qb * 512:(qb + 1) * 512],
                                start=(h2 == 0), stop=(h2 == HPC - 1))
                    yst = ys.tile([128, 1024], f32, name=f"yst_{qb}_{pr}",
                                  tag="yst")
                    nc.scalar.copy(yst[:], psy[:])
                    for i in (0, 1):
                        oc = 2 * pr + i
                        nc.sync.dma_start(
                            out=y_t[oc * 128:(oc + 1) * 128,
                                    qb * 512:(qb + 1) * 512],
                            in_=yst[:, i * 512:(i + 1) * 512])

            for qb in range(NB):
                for h in range(HPC):
                    hp, a = h // 2, h % 2
                    poff = 64 * a
                    pv = pvp.tile([128, 1024], f32, name=f"pv_{h}_{qb}",
                                  tag="pv")
                    sm = smp.tile([128, 1024], f32, name=f"sm_{h}_{qb}",
                                  tag="sm")
                    nkc = 4 * qb + 4
                    pvq = []   # (E, qoff, first, last, kc) pending PV/SM

                    def emit_pv_sm(item):
                        E_, qoff_, first_, last_, kc_ = item
                        for g in (0, 1):
                            sl = slice(g * 512 + qoff_, (g + 1) * 512)
                            nc.tensor.matmul(
                                pv[:, sl],
                                VH[kc_][:, h * 128:(h + 1) * 128],
                                E_[:, sl], start=first_, stop=last_)
                            nc.tensor.matmul(
                                sm[:, sl], ones_a[:], E_[:, sl],
                                start=first_, stop=last_)

                    for kc in range(nkc):
                        j = kc - 4 * qb
                        qoff = max(j, 0) * 128
                        ps = sp.tile([128, 1024], f32, name=f"s_{h}_{qb}_{kc}",
                                     tag="s")
                        for g in (0, 1):
                            tq = 2 * g + hp
                            nc.tensor.matmul(
                                ps[:, g * 512 + qoff:(g + 1) * 512],
                                KT[tq][poff:poff + 64,
                                       kc * 128:(kc + 1) * 128],
                                QT[tq][poff:poff + 64,
                                       qb * 512 + qoff:(qb + 1) * 512],
                                start=True, stop=True)
                        E = ep.tile([128, 1024], bf16, name=f"e_{h}_{qb}_{kc}",
                                    tag="e")
                        if qoff == 0:
                            nc.scalar.activation(E[:], ps[:], Act.Exp,
                                                 scale=0.125)
                        else:
                            for g in (0, 1):
                                nc.scalar.activation(
                                    E[:, g * 512 + qoff:(g + 1) * 512],
                                    ps[:, g * 512 + qoff:(g + 1) * 512],
                                    Act.Exp, scale=0.125)
                        if j >= 0:
                            for g in (0, 1):
                                sl = E[:, g * 512 + qoff:g * 512 + qoff + 128]
                                # causal mask on the idle GpSimd engine (all
                                # SBUF operands): keeps the mask->PV latency
                                # off the vector queue, where the rsqrt chain
                                # flushed at kc==3 would delay qb=1's diagonal
                                # chunks
                                nc.gpsimd.tensor_mul(sl, sl, trimask[:])
                        if kc == 3 and pending_rms:
                            flush_rms()
                        pvq.append((E, qoff, kc == 0, kc == nkc - 1, kc))
                        if len(pvq) > 3:
                            emit_pv_sm(pvq.pop(0))
                    while pvq:
                        emit_pv_sm(pvq.pop(0))
                    # combine: U = pv1/sm1 - lam * pv2/sm2, then RMS norm
                    rb = cb.tile([128, 1024], f32, name=f"rb_{h}_{qb}",
                                 tag="rb")
                    nc.vector.reciprocal_approx_fast(rb[:], sm[:])
                    tt = cb.tile([128, 1024], f32, name=f"tt_{h}_{qb}",
                                 tag="tt")
                    nc.vector.tensor_mul(tt[:], pv[:], rb[:])
                    dst = U[h][:, qb * 512:(qb + 1) * 512]
                    nc.vector.scalar_tensor_tensor(
                        dst, tt[:, 512:1024], neglam[:], tt[:, 0:512],
                        op0=Alu.mult, op1=Alu.add)
                    sq = cb.tile([128, 512], bf16, name=f"sq_{h}_{qb}",
                                 tag="sq")
                    nc.vector.tensor_mul(sq[:], dst, dst)
                    pending_rms.append((h, qb, sq, dst))
                    if h == 0 and pending_y is not None:
                        emit_y(pending_y)
                        pending_y = None
                pending_y = qb
            while pending_rms:
                flush_rms()
            emit_y(pending_y, final=True)

    nc.compile()
    return nc


def _host_prep(q, k, v, Wq, Wk, Wv, Wo, lambda_q1, lambda_k1, lambda_q2,
               lambda_k2, gnorm_w, cos_emb, sin_emb):
    import ml_dtypes

    f32 = np.float32
    bf16 = ml_dtypes.bfloat16
    q = np.asarray(q, f32); k = np.asarray(k, f32); v = np.asarray(v, f32)
    Wq = np.asarray(Wq, f32); Wk = np.asarray(Wk, f32)
    Wv = np.asarray(Wv, f32); Wo = np.asarray(Wo, f32)
    gnorm_w = np.asarray(gnorm_w, f32)
    cos_emb = np.asarray(cos_emb, f32); sin_emb = np.asarray(sin_emb, f32)

    lam1 = np.exp(np.sum(np.asarray(lambda_q1, f32) * np.asarray(lambda_k1, f32),
                         dtype=f32))
    lam2 = np.exp(np.sum(np.asarray(lambda_q2, f32) * np.asarray(lambda_k2, f32),
                         dtype=f32))
    lam = np.float32(lam1 - lam2 + LAMBDA_INIT)

    # per-batch transposed activations (bf16)
    xt = {}
    for b in range(B):
        xt[("q", b)] = np.ascontiguousarray(q[b].T).astype(bf16)
        xt[("k", b)] = np.ascontiguousarray(k[b].T).astype(bf16)
        xt[("v", b)] = np.ascontiguousarray(v[b].T).astype(bf16)

    # shared constant tensors
    base_c = cos_emb[:S, :QKD]          # [S, 64]
    base_s = sin_emb[:S, :QKD]
    crep = np.ascontiguousarray(np.tile(base_c.T, (2, 1))).astype(bf16)
    srep = np.ascontiguousarray(np.tile(base_s.T, (2, 1))).astype(bf16)
    pmat = np.zeros((128, 128), f32)
    for blk in range(2):
        o = blk * 64
        for i in range(QKD // 2):
            pmat[o + 2 * i, o + 2 * i + 1] = 1.0     # lhsT[2i, 2i+1]
            pmat[o + 2 * i + 1, o + 2 * i] = -1.0    # lhsT[2i+1, 2i]
    pmat = pmat.astype(bf16)
    ones_a = np.ones((128, 128), f32).astype(bf16)
    # trimask[p, n] = 1 if key-in-chunk p <= query-in-block n (valid)
    trimask = np.triu(np.ones((128, 128), f32), 0).astype(bf16)
    neglam = np.full((128, 1), -lam, f32)

    per_core = []
    for c in range(NCORES):
        b, grp = c // GRPS, c % GRPS
        heads = [HPC * grp + j for j in range(HPC)]
        # wq/wk columns: tile t = 2*g + hp; within tile: head 2*hp+a at
        # cols [64*a, 64*a+64), original feature order (interleaved pairs)
        cols = []
        for t in range(4):
            g, hp = t // 2, t % 2
            for a2 in range(2):
                hg = heads[2 * hp + a2]
                cols.extend(hg * HD + g * QKD + d for d in range(QKD))
        cols = np.asarray(cols)
        vrows = np.asarray([h * HD + d for h in heads for d in range(HD)])
        wq_t = np.ascontiguousarray(Wq[cols, :].T).astype(bf16)
        wk_t = np.ascontiguousarray(Wk[cols, :].T).astype(bf16)
        wv_t = np.ascontiguousarray(Wv[vrows, :].T).astype(bf16)
        gtile = np.tile(gnorm_w, HPC)                       # [512]
        wo_t = np.ascontiguousarray(
            ((1.0 - LAMBDA_INIT) * Wo[:, vrows] * gtile[None, :]).T).astype(bf16)
        per_core.append({
            "xq_t": xt[("q", b)], "xk_t": xt[("k", b)], "xv_t": xt[("v", b)],
            "wq_t": wq_t, "wk_t": wk_t, "wv_t": wv_t, "wo_t": wo_t,
            "crep": crep, "srep": srep, "pmat": pmat,
            "ones_a": ones_a, "trimask": trimask, "neglam": neglam,
        })
    return per_core


def _install_ntff_hook():
    """antenv.axon_hooks is absent in this image; synthesize it so
    run_bass_kernel_spmd(trace=True) can capture NTFF profiles."""
    import sys as _sys
    import types

    if "antenv.axon_hooks" in _sys.modules:
        return
    import antenv
    mod = types.ModuleType("antenv.axon_hooks")
    state = {"hook": None}
    mod.set_axon_ntff_profile_hook = lambda h: state.__setitem__("hook", h)
    mod.get_axon_ntff_profile_hook = lambda: state["hook"]
    _sys.modules["antenv.axon_hooks"] = mod
    antenv.axon_hooks = mod
    try:
        from trn_agent_boot.trn_boot import _ntff_profile_via_ctypes
        state["hook"] = _ntff_profile_via_ctypes("/opt/axon/libaxon_pjrt.so")
    except Exception as e:  # degrade: trace skipped, run still works
        print("ntff hook install failed:", e)


def kernel(q, k, v, Wq, Wk, Wv, Wo, lambda_q1, lambda_k1, lambda_q2,
           lambda_k2, gnorm_w, cos_emb, sin_emb, mask, _trace=False):
    if _trace:
        _install_ntff_hook()
    global _PROGRAM
    if _PROGRAM is None:
        _PROGRAM = _build_program()
    nc = _PROGRAM

    in_maps = _host_prep(q, k, v, Wq, Wk, Wv, Wo, lambda_q1, lambda_k1,
                         lambda_q2, lambda_k2, gnorm_w, cos_emb, sin_emb)

    from concourse.bass_utils import run_bass_kernel_spmd
    res = run_bass_kernel_spmd(nc, in_maps, core_ids=list(range(NCORES)),
                               trace=_trace)
    kernel.last_result = res

    y = np.zeros((B, S, HID), np.float32)
    for c in range(NCORES):
        y[c // GRPS] += res.results[c]["y_t"].T
    return y
